# revision 29
# baseline (speedup 1.0000x reference)
"""Multi-head attention (B=4, S=2048, D=1024, H=16, E=64) on 8 TRN2 cores.

Sharding: tensor-parallel over heads — 2 heads per core. Each core gets the
full token stream x^T plus its 2 heads' Wq/Wk/Wv, computes its [T, 128] slice
of the concatenated output, and the host stitches the 8 slices together.

Per-core dataflow (all SBUF-resident between the x load and the out store):
  qT/kT = W @ x^T            [he=128, T]   (bf16 matmul, fp32 accum)
  vT    = W_v @ x^T  --PE-transpose-->  v_aug[t, e | ones]  (ones col -> Z)
  scoresT[j, i] = k @ q^T    (fp32r, K=64; both heads auto row-packed via
                              base_partition -> concurrent PE subarrays)
  expT = exp(scoresT / 8)    (ACT, softmax max-subtraction skipped: |scores|
                              is bounded ~±1.5 so exp cannot overflow)
  av[e|Z, i] = v_aug^T @ expT  accumulated over j-tiles in PSUM; row 64 is
                              the softmax denominator Z_i
  out[i, e] = transpose(av) * (1/Z)  --> DMA out
"""

import numpy as np
import ml_dtypes

import concourse.bass as bass
import concourse.mybir as mybir
import concourse.tile as tile

# ---------------------------------------------------------------------------
# This walrus build rejects instructions carrying more than one sync wait
# ("Too many sync wait commands", CoreV3GenImpl.cpp:104). After Tile
# scheduling, move excess waits onto single-wait NOPs inserted just before
# the instruction on the same engine queue — sequential waits on the queue
# are semantically identical to one multi-wait instruction.
_MAX_WAITS = 1
_ws_counter = [0]


def split_sync_waits(nc, max_waits=_MAX_WAITS):
    for fn in nc.m.functions:
        for bb in fn.blocks:
            insts = bb.instructions
            if not any(
                i.sync_info is not None and len(i.sync_info.on_wait) > max_waits
                for i in insts
            ):
                continue
            new = []
            for inst in insts:
                si = inst.sync_info
                waits = list(si.on_wait) if si is not None else []
                if len(waits) > max_waits:
                    excess, keep = waits[:-max_waits], waits[-max_waits:]
                    while excess:
                        chunk, excess = excess[:max_waits], excess[max_waits:]
                        nop = mybir.InstNoOp(name=f"I-wsplit-{_ws_counter[0]}")
                        _ws_counter[0] += 1
                        nop.engine = inst.engine
                        nop.sync_info = mybir.SyncInfo(on_wait=chunk, on_update=[])
                        new.append(nop)
                    inst.sync_info = mybir.SyncInfo(
                        on_wait=keep, on_update=list(si.on_update)
                    )
                new.append(inst)
            bb.instructions = new
# ---------------------------------------------------------------------------

F32 = mybir.dt.float32
F32R = mybir.dt.float32r
BF16 = mybir.dt.bfloat16
EXP = mybir.ActivationFunctionType.Exp

N_CORES = 8


class Cfg:
    def __init__(self, B=4, S=2048, D=1024, E=64, HPC=2):
        self.B, self.S, self.D, self.E, self.HPC = B, S, D, E, HPC
        self.T = B * S                 # total tokens
        self.M = HPC * E               # packed head-dim cols per core (=128)
        self.NC = min(512, S)          # i/t chunk (psum free size)
        self.JT = 128                  # j tile (partition dim)
        self.DT = 128                  # d tile (contraction)
        self.n_d = self.D // self.DT   # d tiles
        self.n_tc = self.S // self.NC  # chunks per batch (t and i)
        self.n_jt = self.S // self.JT  # j tiles per batch
        self.n_i2 = self.NC // 128     # 128-rows per chunk
        assert self.M == 128


FULL = Cfg()


def build(cfg: Cfg = FULL, split_waits: bool = True, act_cycle_ns: float | None = 1.1):
    B, S, D, E = cfg.B, cfg.S, cfg.D, cfg.E
    T, M, NC, JT = cfg.T, cfg.M, cfg.NC, cfg.JT
    VW = E + 1  # v_aug cols per (head, j-tile): E values + ones
    VP = 80     # padded to a 16-multiple (DMA xbar transpose granularity);
                # cols VW..VP-1 stay zero so the padded av rows are defined

    if act_cycle_ns is not None:
        # The Tile scheduler's cost model under-estimates ACTIVATE (measured
        # 1114ns for [128,1024] exp vs ~1050 modeled) and models LDWEIGHTS as
        # free, so its static order under-fills the PE during the ACT-bound
        # attention phase and dumps the QKV filler at batch boundaries.
        # Inflating the modeled ACT cycle makes the scheduler weave more PE
        # filler inline. Only exp runs on ACT here, so nothing else shifts.
        from concourse import hw_specs
        hw_specs.TRN2Spec.CYCLE_T[mybir.EngineType.Activation] = act_cycle_ns

    nc = bass.Bass()
    xt = nc.declare_dram_parameter("xt", [D, T], BF16, isOutput=False)
    wq = nc.declare_dram_parameter("wq", [D, M], BF16, isOutput=False)
    wk = nc.declare_dram_parameter("wk", [D, M], BF16, isOutput=False)
    wv = nc.declare_dram_parameter("wv", [D, M], BF16, isOutput=False)
    out = nc.declare_dram_parameter("out", [T, M], F32, isOutput=True)

    with tile.TileContext(nc) as tc:
        # ---- pools -------------------------------------------------------
        with (
            tc.tile_pool(name="statics", bufs=1) as static_pool,
            tc.tile_pool(name="xt_pool", bufs=48) as xt_pool,
            tc.tile_pool(name="stage", bufs=3) as stage_pool,
            tc.tile_pool(name="expp", bufs=6) as exp_pool,
            tc.tile_pool(name="avsb", bufs=4) as avsb_pool,
            tc.tile_pool(name="outp", bufs=6) as out_pool,
            tc.tile_pool(name="smallp", bufs=8) as small_pool,
            tc.tile_pool(name="psA", bufs=1, space="PSUM") as psA,   # qkv + v-transposes
            tc.tile_pool(name="psS", bufs=2, space="PSUM") as psS,   # scoresT
            tc.tile_pool(name="psV", bufs=3, space="PSUM") as psV,   # av accum + out-transposes
        ):
            # ---- static SBUF tensors -------------------------------------
            qT_sb = static_pool.tile([128, T], BF16, name="qT_sb", tag="qT_sb")
            kT_sb = static_pool.tile([128, T], BF16, name="kT_sb", tag="kT_sb")
            # v_aug: slice (h, gjt) at cols [(h*(T//JT) + gjt)*VW : +VW]
            n_gjt = T // JT
            v_sb = static_pool.tile(
                [128, cfg.HPC * n_gjt * VP], BF16, name="v_sb", tag="v_sb"
            )
            ident = static_pool.tile([128, 128], F32, name="ident", tag="ident")
            w_sb = {}
            for nm, srcp in (("q", wq), ("k", wk), ("v", wv)):
                w_sb[nm] = static_pool.tile(
                    [128, cfg.n_d * M], BF16, name=f"w{nm}_sb", tag=f"w{nm}_sb"
                )
                for d in range(cfg.n_d):
                    nc.sync.dma_start(
                        out=w_sb[nm][:, d * M:(d + 1) * M],
                        in_=srcp[d * 128:(d + 1) * 128, :],
                    )

            # identity matrices for PE transposes (dtype must match input)
            from concourse.masks import make_identity
            make_identity(nc, ident[:, :])
            ident_bf = static_pool.tile([128, 128], BF16, name="ident_bf", tag="ident_bf")
            make_identity(nc, ident_bf[:, :])

            # v_aug init: zero the padding, then the ones column at local
            # col E of every VP-block (strided memset fails walrus's ISA
            # check, so broadcast-copy from a constant instead).
            nc.gpsimd.memset(v_sb[:, :], 0.0)
            ones_c = static_pool.tile([128, 1], F32, name="ones_c", tag="ones_c")
            nc.vector.memset(ones_c[:, :], 1.0)
            n_slots = cfg.HPC * n_gjt
            ones_view = v_sb.rearrange("p (g c) -> p g c", c=VP)[:, :, E:E + 1]
            nc.vector.tensor_copy(ones_view, ones_c.broadcast_to((128, n_slots, 1)))

            from contextlib import contextmanager

            @contextmanager
            def low_priority(off=1_000_000):
                """Filler work (QKV for the next batch, output finishing for
                the previous chunk) must never be dispatched ahead of the
                ACT-critical sc->exp->av chain; push it behind in the
                scheduler's priority heap."""
                saved = tc.cur_priority
                tc.cur_priority = saved + off
                try:
                    yield
                finally:
                    tc.cur_priority = saved

            def emit_xt_loads(b):
                """Per-(d, chunk) tiles, chunk-major, so the first QKV groups
                (and with them the first attention chunk) start after ~1MB of
                DMA instead of the batch's full 4MB."""
                t0 = b * S
                xts = {}
                for c in range(cfg.n_tc):
                    for d in range(cfg.n_d):
                        xti = xt_pool.tile([128, NC], BF16, name="xti", tag="xti")
                        nc.sync.dma_start(
                            out=xti[:, :],
                            in_=xt[
                                d * 128:(d + 1) * 128,
                                t0 + c * NC: t0 + (c + 1) * NC,
                            ],
                        )
                        xts[(d, c)] = xti
                return xts

            def qkv_stream(b, xts, lp=True, order=None):
                """Yield one fine-grained QKV work item at a time (so the
                in-order PE queue never gets a long filler block)."""
                t0 = b * S
                ctx = low_priority if lp else _null_lp
                if order is None:
                    order = [(nm, c) for c in range(cfg.n_tc) for nm in ("q", "k", "v")]
                for nm, c in order:
                    if True:
                        dst = {"q": qT_sb, "k": kT_sb, "v": None}[nm]
                        with ctx():
                            ps = psA.tile([128, NC], F32, name="qkv_ps", tag="psA")
                        for d in range(cfg.n_d):
                            with ctx():
                                nc.tensor.matmul(
                                    ps[:, :],
                                    lhsT=w_sb[nm][:, d * M:(d + 1) * M],
                                    rhs=xts[(d, c)][:, :],
                                    start=(d == 0),
                                    stop=(d == cfg.n_d - 1),
                                )
                            yield
                        if dst is not None:
                            with ctx():
                                nc.vector.tensor_copy(
                                    dst[:, t0 + c * NC: t0 + (c + 1) * NC], ps[:, :]
                                )
                            yield
                        else:
                            with ctx():
                                vst = stage_pool.tile([128, NC], BF16, name="vst")
                                nc.vector.tensor_copy(vst[:, :], ps[:, :])
                            yield
                            for i2 in range(cfg.n_i2):
                                with ctx():
                                    tp = psA.tile(
                                        [128, 128], BF16, name="vtp", tag="psA"
                                    )
                                    nc.tensor.transpose(
                                        tp[:, :],
                                        vst[:, i2 * 128:(i2 + 1) * 128],
                                        ident_bf[:, :],
                                    )
                                    gjt = b * cfg.n_jt + c * cfg.n_i2 + i2
                                    for h in range(cfg.HPC):
                                        base = (h * n_gjt + gjt) * VP
                                        nc.vector.tensor_copy(
                                            v_sb[:, base:base + E],
                                            tp[:, h * E:(h + 1) * E],
                                        )
                                yield

            def out_stream(b, ic, avs):
                """Yield the normalize+transpose+store work for one finished
                ic chunk, one transpose at a time."""
                t0 = b * S
                for i2 in range(cfg.n_i2):
                    with low_priority():
                        ot = out_pool.tile([128, M], F32, name="ot", tag="ot")
                    for h in range(cfg.HPC):
                        with low_priority():
                            tp = psV.tile([128, VW], BF16, name="otp", tag="psV")
                            nc.tensor.transpose(
                                tp[:, :],
                                avs[h][:VW, i2 * 128:(i2 + 1) * 128],
                                ident_bf[:VW, :VW],
                            )
                            rec = small_pool.tile([128, 1], F32, name="rec", tag="rec")
                            nc.vector.reciprocal(rec[:, :], tp[:, E:E + 1])
                            nc.vector.tensor_scalar_mul(
                                ot[:, h * E:(h + 1) * E], tp[:, :E], rec[:, :]
                            )
                        yield
                    with low_priority():
                        row = t0 + ic * NC + i2 * 128
                        nc.sync.dma_start(out=out[row:row + 128, :], in_=ot[:, :])
                    yield

            def drain_n(stream, n):
                if stream is None:
                    return None
                for _ in range(n):
                    try:
                        next(stream)
                    except StopIteration:
                        return None
                return stream

            def drain_all(stream):
                if stream is not None:
                    for _ in stream:
                        pass

            def emit_attention_jts(b, ic, fillers):
                """The ACT-bound jt loop; after each jt, pull a few items from
                the filler streams to keep the PE queue dense but never
                blocked. Returns the finished av_sb pair."""
                t0 = b * S
                av_ps = [
                    psV.tile([VP, NC], F32, name=f"av_ps{h}", tag="psV")
                    for h in range(cfg.HPC)
                ]
                def emit_av_pair(jt, ex):
                    gjt = b * cfg.n_jt + jt
                    for h in range(cfg.HPC):
                        base = (h * n_gjt + gjt) * VP
                        nc.tensor.matmul(
                            av_ps[h][:, :],
                            lhsT=v_sb[:, base:base + VP],
                            rhs=ex[:JT, h * NC:(h + 1) * NC],
                            start=(jt == 0),
                            stop=(jt == cfg.n_jt - 1),
                        )

                av_prev = None
                for jt in range(cfg.n_jt):
                    sc = psS.tile([128, cfg.HPC * NC], F32, name="sc_ps", tag="psS")
                    for h in range(cfg.HPC):
                        nc.tensor.matmul(
                            sc[:JT, h * NC:(h + 1) * NC],
                            lhsT=kT_sb[
                                h * E:(h + 1) * E,
                                t0 + jt * JT: t0 + (jt + 1) * JT,
                            ],
                            rhs=qT_sb[
                                h * E:(h + 1) * E,
                                t0 + ic * NC: t0 + (ic + 1) * NC,
                            ],
                            start=True,
                            stop=True,
                        )
                    ex = exp_pool.tile([128, cfg.HPC * NC], BF16, name="ex", tag="ex")
                    nc.scalar.activation(
                        ex[:JT, :], sc[:JT, :], EXP, scale=1.0 / np.sqrt(E)
                    )
                    if av_prev is not None:
                        emit_av_pair(*av_prev)
                        for si in range(len(fillers)):
                            fillers[si] = drain_n(fillers[si], 2)
                    av_prev = (jt, ex)
                emit_av_pair(*av_prev)

                avs = []
                for h in range(cfg.HPC):
                    av_sb = avsb_pool.tile([VP, NC], BF16, name="av_sb", tag="av_sb")
                    nc.vector.tensor_copy(av_sb[:, :], av_ps[h][:, :])
                    avs.append(av_sb)
                return avs

            # Schedule: QKV(0) up front; then for each batch, run the
            # ACT-bound jt loops with (a) prev chunk's out-stage and (b) next
            # batch's QKV interleaved between jts as PE filler.
            from contextlib import nullcontext

            def _null_lp():
                return nullcontext()

            # Batch 0 ramp: attention(0, ic0) consumes one k/v chunk every
            # 4 jt (~4.5us) but q chunks 1-3 aren't needed until ic1 — emit
            # them last so the k/v chunks keep pace with the jt loop.
            ramp_order = (
                [("q", 0), ("k", 0), ("v", 0)]
                + [(nm, c) for c in range(1, cfg.n_tc) for nm in ("k", "v")]
                + [("q", c) for c in range(1, cfg.n_tc)]
            )
            xts_cur = emit_xt_loads(0)
            drain_all(qkv_stream(0, xts_cur, lp=False, order=ramp_order))
            pending_out = None
            for b in range(B):
                xts_next = emit_xt_loads(b + 1) if b + 1 < B else None
                qs = qkv_stream(b + 1, xts_next) if xts_next is not None else None
                for ic in range(cfg.n_tc):
                    fillers = [pending_out, qs]
                    avs = emit_attention_jts(b, ic, fillers)
                    pending_out, qs = fillers
                    drain_all(pending_out)  # anything left from prev chunk
                    pending_out = out_stream(b, ic, avs)
                xts_cur = xts_next
                # spread remaining qkv work into the next batch boundary
                drain_all(qs)
            drain_all(pending_out)

    if split_waits:
        # walrus needs this; CoreSim chokes on the bare NOPs, so skip there
        split_sync_waits(nc)
    return nc


# ---------------------------------------------------------------------------
# Host entry point
# ---------------------------------------------------------------------------
_CACHE = {}


def _get_nc():
    if "nc" not in _CACHE:
        _CACHE["nc"] = build(FULL)
    return _CACHE["nc"]


def kernel(x: np.ndarray, W_q: np.ndarray, W_k: np.ndarray, W_v: np.ndarray) -> np.ndarray:
    from concourse.bass_utils import run_bass_kernel_spmd

    cfg = FULL
    B, S, D, E, HPC = cfg.B, cfg.S, cfg.D, cfg.E, cfg.HPC
    T, M = cfg.T, cfg.M
    H = HPC * N_CORES

    assert x.shape == (B, S, D) and W_q.shape == (H, E, D)

    bf16 = ml_dtypes.bfloat16
    xtT = np.ascontiguousarray(x.reshape(T, D).T).astype(bf16)  # [D, T]

    in_maps = []
    for c in range(N_CORES):
        m = {"xt": xtT}
        for nm, W in (("wq", W_q), ("wk", W_k), ("wv", W_v)):
            # pack 2 heads: [D, 128] with col h*E+e = W[2c+h][e, :]
            blk = W[HPC * c:HPC * (c + 1)]          # [HPC, E, D]
            wt = blk.reshape(M, D).T                # [D, M]
            m[nm] = np.ascontiguousarray(wt).astype(bf16)
        in_maps.append(m)

    kw = dict(_CACHE.get("run_kwargs", {}))
    res = run_bass_kernel_spmd(_get_nc(), in_maps, list(range(N_CORES)), **kw)
    _CACHE["last_res"] = res
    outs = [res.results[c]["out"] for c in range(N_CORES)]  # each [T, M]
    full = np.concatenate(outs, axis=1)                     # [T, H*E]
    return full.reshape(B, S, H * E).astype(np.float32, copy=False)


# revision 30
# speedup vs baseline: 1.0361x; 1.0361x over previous
"""Multi-head attention (B=4, S=2048, D=1024, H=16, E=64) on 8 TRN2 cores.

Sharding: tensor-parallel over heads — 2 heads per core. Each core gets the
full token stream x^T plus its 2 heads' Wq/Wk/Wv, computes its [T, 128] slice
of the concatenated output, and the host stitches the 8 slices together.

Per-core dataflow (all SBUF-resident between the x load and the out store):
  qT/kT = W @ x^T            [he=128, T]   (bf16 matmul, fp32 accum)
  vT    = W_v @ x^T  --PE-transpose-->  v_aug[t, e | ones]  (ones col -> Z)
  scoresT[j, i] = k @ q^T    (fp32r, K=64; both heads auto row-packed via
                              base_partition -> concurrent PE subarrays)
  expT = exp(scoresT / 8)    (ACT, softmax max-subtraction skipped: |scores|
                              is bounded ~±1.5 so exp cannot overflow)
  av[e|Z, i] = v_aug^T @ expT  accumulated over j-tiles in PSUM; row 64 is
                              the softmax denominator Z_i
  out[i, e] = transpose(av) * (1/Z)  --> DMA out
"""

import numpy as np
import ml_dtypes

import concourse.bass as bass
import concourse.mybir as mybir
import concourse.tile as tile

# ---------------------------------------------------------------------------
# This walrus build rejects instructions carrying more than one sync wait
# ("Too many sync wait commands", CoreV3GenImpl.cpp:104). After Tile
# scheduling, move excess waits onto single-wait NOPs inserted just before
# the instruction on the same engine queue — sequential waits on the queue
# are semantically identical to one multi-wait instruction.
_MAX_WAITS = 1
_ws_counter = [0]


def split_sync_waits(nc, max_waits=_MAX_WAITS):
    for fn in nc.m.functions:
        for bb in fn.blocks:
            insts = bb.instructions
            if not any(
                i.sync_info is not None and len(i.sync_info.on_wait) > max_waits
                for i in insts
            ):
                continue
            new = []
            for inst in insts:
                si = inst.sync_info
                waits = list(si.on_wait) if si is not None else []
                if len(waits) > max_waits:
                    excess, keep = waits[:-max_waits], waits[-max_waits:]
                    while excess:
                        chunk, excess = excess[:max_waits], excess[max_waits:]
                        nop = mybir.InstNoOp(name=f"I-wsplit-{_ws_counter[0]}")
                        _ws_counter[0] += 1
                        nop.engine = inst.engine
                        nop.sync_info = mybir.SyncInfo(on_wait=chunk, on_update=[])
                        new.append(nop)
                    inst.sync_info = mybir.SyncInfo(
                        on_wait=keep, on_update=list(si.on_update)
                    )
                new.append(inst)
            bb.instructions = new
# ---------------------------------------------------------------------------

F32 = mybir.dt.float32
F32R = mybir.dt.float32r
BF16 = mybir.dt.bfloat16
EXP = mybir.ActivationFunctionType.Exp

N_CORES = 8


class Cfg:
    def __init__(self, B=4, S=2048, D=1024, E=64, HPC=2):
        self.B, self.S, self.D, self.E, self.HPC = B, S, D, E, HPC
        self.T = B * S                 # total tokens
        self.M = HPC * E               # packed head-dim cols per core (=128)
        self.NC = min(512, S)          # i/t chunk (psum free size)
        self.JT = 128                  # j tile (partition dim)
        self.DT = 128                  # d tile (contraction)
        self.n_d = self.D // self.DT   # d tiles
        self.n_tc = self.S // self.NC  # chunks per batch (t and i)
        self.n_jt = self.S // self.JT  # j tiles per batch
        self.n_i2 = self.NC // 128     # 128-rows per chunk
        assert self.M == 128


FULL = Cfg()


def build(cfg: Cfg = FULL, split_waits: bool = True, act_cycle_ns: float | None = 1.1):
    B, S, D, E = cfg.B, cfg.S, cfg.D, cfg.E
    T, M, NC, JT = cfg.T, cfg.M, cfg.NC, cfg.JT
    VW = E + 1  # v_aug cols per (head, j-tile): E values + ones
    VP = 80     # padded to a 16-multiple (DMA xbar transpose granularity);
                # cols VW..VP-1 stay zero so the padded av rows are defined

    if act_cycle_ns is not None:
        # The Tile scheduler's cost model under-estimates ACTIVATE (measured
        # 1114ns for [128,1024] exp vs ~1050 modeled) and models LDWEIGHTS as
        # free, so its static order under-fills the PE during the ACT-bound
        # attention phase and dumps the QKV filler at batch boundaries.
        # Inflating the modeled ACT cycle makes the scheduler weave more PE
        # filler inline. Only exp runs on ACT here, so nothing else shifts.
        from concourse import hw_specs
        hw_specs.TRN2Spec.CYCLE_T[mybir.EngineType.Activation] = act_cycle_ns

    nc = bass.Bass()
    xt = nc.declare_dram_parameter("xt", [D, T], BF16, isOutput=False)
    wq = nc.declare_dram_parameter("wq", [D, M], BF16, isOutput=False)
    wk = nc.declare_dram_parameter("wk", [D, M], BF16, isOutput=False)
    wv = nc.declare_dram_parameter("wv", [D, M], BF16, isOutput=False)
    out = nc.declare_dram_parameter("out", [T, M], F32, isOutput=True)

    with tile.TileContext(nc) as tc:
        # ---- pools -------------------------------------------------------
        with (
            tc.tile_pool(name="statics", bufs=1) as static_pool,
            tc.tile_pool(name="xt_pool", bufs=24) as xt_pool,
            tc.tile_pool(name="stage", bufs=3) as stage_pool,
            tc.tile_pool(name="expp", bufs=6) as exp_pool,
            tc.tile_pool(name="avsb", bufs=4) as avsb_pool,
            tc.tile_pool(name="outp", bufs=6) as out_pool,
            tc.tile_pool(name="smallp", bufs=8) as small_pool,
            tc.tile_pool(name="psA", bufs=1, space="PSUM") as psA,   # qkv + v-transposes
            tc.tile_pool(name="psS", bufs=2, space="PSUM") as psS,   # scoresT
            tc.tile_pool(name="psV", bufs=3, space="PSUM") as psV,   # av accum + out-transposes
        ):
            # ---- static SBUF tensors -------------------------------------
            qT_sb = static_pool.tile([128, T], BF16, name="qT_sb", tag="qT_sb")
            kT_sb = static_pool.tile([128, T], BF16, name="kT_sb", tag="kT_sb")
            # v_aug: slice (h, gjt) at cols [(h*(T//JT) + gjt)*VW : +VW]
            n_gjt = T // JT
            v_sb = static_pool.tile(
                [128, cfg.HPC * n_gjt * VP], BF16, name="v_sb", tag="v_sb"
            )
            ident = static_pool.tile([128, 128], F32, name="ident", tag="ident")
            w_sb = {}
            for nm, srcp in (("q", wq), ("k", wk), ("v", wv)):
                w_sb[nm] = static_pool.tile(
                    [128, cfg.n_d * M], BF16, name=f"w{nm}_sb", tag=f"w{nm}_sb"
                )
                # single DMA: [d, m] -> partition d%128, col (d//128)*M + m
                nc.sync.dma_start(
                    out=w_sb[nm].rearrange("p (dt m) -> p dt m", dt=cfg.n_d),
                    in_=srcp.rearrange("(dt p) m -> p dt m", p=128),
                )

            # identity matrices for PE transposes (dtype must match input)
            from concourse.masks import make_identity
            make_identity(nc, ident[:, :])
            ident_bf = static_pool.tile([128, 128], BF16, name="ident_bf", tag="ident_bf")
            make_identity(nc, ident_bf[:, :])

            # v_aug init: zero the padding, then the ones column at local
            # col E of every VP-block (strided memset fails walrus's ISA
            # check, so broadcast-copy from a constant instead).
            nc.gpsimd.memset(v_sb[:, :], 0.0)
            ones_c = static_pool.tile([128, 1], F32, name="ones_c", tag="ones_c")
            nc.vector.memset(ones_c[:, :], 1.0)
            n_slots = cfg.HPC * n_gjt
            ones_view = v_sb.rearrange("p (g c) -> p g c", c=VP)[:, :, E:E + 1]
            nc.vector.tensor_copy(ones_view, ones_c.broadcast_to((128, n_slots, 1)))

            from contextlib import contextmanager

            @contextmanager
            def low_priority(off=1_000_000):
                """Filler work (QKV for the next batch, output finishing for
                the previous chunk) must never be dispatched ahead of the
                ACT-critical sc->exp->av chain; push it behind in the
                scheduler's priority heap."""
                saved = tc.cur_priority
                tc.cur_priority = saved + off
                try:
                    yield
                finally:
                    tc.cur_priority = saved

            def emit_xt_loads(b):
                """Per-(d, half-batch) tiles, half-major: big enough that the
                sync sequencer's ~650ns per-DMA issue cost doesn't serialize
                the ramp, small enough that the first QKV chunk starts after
                2MB instead of the batch's full 4MB."""
                t0 = b * S
                xts = {}
                for ch in range(cfg.n_tc // 2):
                    for d in range(cfg.n_d):
                        xti = xt_pool.tile([128, 2 * NC], BF16, name="xti", tag="xti")
                        nc.sync.dma_start(
                            out=xti[:, :],
                            in_=xt[
                                d * 128:(d + 1) * 128,
                                t0 + ch * 2 * NC: t0 + (ch + 1) * 2 * NC,
                            ],
                        )
                        xts[(d, ch)] = xti
                return xts

            def qkv_stream(b, xts, lp=True, order=None):
                """Yield one fine-grained QKV work item at a time (so the
                in-order PE queue never gets a long filler block)."""
                t0 = b * S
                ctx = low_priority if lp else _null_lp
                if order is None:
                    order = [(nm, c) for c in range(cfg.n_tc) for nm in ("q", "k", "v")]
                for nm, c in order:
                    if True:
                        dst = {"q": qT_sb, "k": kT_sb, "v": None}[nm]
                        with ctx():
                            ps = psA.tile([128, NC], F32, name="qkv_ps", tag="psA")
                        for d in range(cfg.n_d):
                            with ctx():
                                nc.tensor.matmul(
                                    ps[:, :],
                                    lhsT=w_sb[nm][:, d * M:(d + 1) * M],
                                    rhs=xts[(d, c // 2)][:, (c % 2) * NC:(c % 2 + 1) * NC],
                                    start=(d == 0),
                                    stop=(d == cfg.n_d - 1),
                                )
                            yield
                        if dst is not None:
                            with ctx():
                                nc.vector.tensor_copy(
                                    dst[:, t0 + c * NC: t0 + (c + 1) * NC], ps[:, :]
                                )
                            yield
                        else:
                            with ctx():
                                vst = stage_pool.tile([128, NC], BF16, name="vst")
                                nc.vector.tensor_copy(vst[:, :], ps[:, :])
                            yield
                            for i2 in range(cfg.n_i2):
                                with ctx():
                                    tp = psA.tile(
                                        [128, 128], BF16, name="vtp", tag="psA"
                                    )
                                    nc.tensor.transpose(
                                        tp[:, :],
                                        vst[:, i2 * 128:(i2 + 1) * 128],
                                        ident_bf[:, :],
                                    )
                                    gjt = b * cfg.n_jt + c * cfg.n_i2 + i2
                                    for h in range(cfg.HPC):
                                        base = (h * n_gjt + gjt) * VP
                                        nc.vector.tensor_copy(
                                            v_sb[:, base:base + E],
                                            tp[:, h * E:(h + 1) * E],
                                        )
                                yield

            def out_stream(b, ic, avs):
                """Yield the normalize+transpose+store work for one finished
                ic chunk, one transpose at a time."""
                t0 = b * S
                for i2 in range(cfg.n_i2):
                    with low_priority():
                        ot = out_pool.tile([128, M], F32, name="ot", tag="ot")
                    for h in range(cfg.HPC):
                        with low_priority():
                            tp = psV.tile([128, VW], BF16, name="otp", tag="psV")
                            nc.tensor.transpose(
                                tp[:, :],
                                avs[h][:VW, i2 * 128:(i2 + 1) * 128],
                                ident_bf[:VW, :VW],
                            )
                            rec = small_pool.tile([128, 1], F32, name="rec", tag="rec")
                            nc.vector.reciprocal(rec[:, :], tp[:, E:E + 1])
                            nc.vector.tensor_scalar_mul(
                                ot[:, h * E:(h + 1) * E], tp[:, :E], rec[:, :]
                            )
                        yield
                    with low_priority():
                        row = t0 + ic * NC + i2 * 128
                        nc.sync.dma_start(out=out[row:row + 128, :], in_=ot[:, :])
                    yield

            def drain_n(stream, n):
                if stream is None:
                    return None
                for _ in range(n):
                    try:
                        next(stream)
                    except StopIteration:
                        return None
                return stream

            def drain_all(stream):
                if stream is not None:
                    for _ in stream:
                        pass

            def emit_attention_jts(b, ic, fillers):
                """The ACT-bound jt loop; after each jt, pull a few items from
                the filler streams to keep the PE queue dense but never
                blocked. Returns the finished av_sb pair."""
                t0 = b * S
                av_ps = [
                    psV.tile([VP, NC], F32, name=f"av_ps{h}", tag="psV")
                    for h in range(cfg.HPC)
                ]
                def emit_av_pair(jt, ex):
                    gjt = b * cfg.n_jt + jt
                    for h in range(cfg.HPC):
                        base = (h * n_gjt + gjt) * VP
                        nc.tensor.matmul(
                            av_ps[h][:, :],
                            lhsT=v_sb[:, base:base + VP],
                            rhs=ex[:JT, h * NC:(h + 1) * NC],
                            start=(jt == 0),
                            stop=(jt == cfg.n_jt - 1),
                        )

                av_prev = None
                for jt in range(cfg.n_jt):
                    sc = psS.tile([128, cfg.HPC * NC], F32, name="sc_ps", tag="psS")
                    for h in range(cfg.HPC):
                        nc.tensor.matmul(
                            sc[:JT, h * NC:(h + 1) * NC],
                            lhsT=kT_sb[
                                h * E:(h + 1) * E,
                                t0 + jt * JT: t0 + (jt + 1) * JT,
                            ],
                            rhs=qT_sb[
                                h * E:(h + 1) * E,
                                t0 + ic * NC: t0 + (ic + 1) * NC,
                            ],
                            start=True,
                            stop=True,
                        )
                    ex = exp_pool.tile([128, cfg.HPC * NC], BF16, name="ex", tag="ex")
                    nc.scalar.activation(
                        ex[:JT, :], sc[:JT, :], EXP, scale=1.0 / np.sqrt(E)
                    )
                    if av_prev is not None:
                        emit_av_pair(*av_prev)
                        for si in range(len(fillers)):
                            fillers[si] = drain_n(fillers[si], 2)
                    av_prev = (jt, ex)
                emit_av_pair(*av_prev)

                avs = []
                for h in range(cfg.HPC):
                    av_sb = avsb_pool.tile([VP, NC], BF16, name="av_sb", tag="av_sb")
                    nc.vector.tensor_copy(av_sb[:, :], av_ps[h][:, :])
                    avs.append(av_sb)
                return avs

            # Schedule: QKV(0) up front; then for each batch, run the
            # ACT-bound jt loops with (a) prev chunk's out-stage and (b) next
            # batch's QKV interleaved between jts as PE filler.
            from contextlib import nullcontext

            def _null_lp():
                return nullcontext()

            # Batch 0 ramp: attention(0, ic0) consumes one k/v chunk every
            # 4 jt (~4.5us) but q chunks 1-3 aren't needed until ic1 — emit
            # them last so the k/v chunks keep pace with the jt loop.
            ramp_order = (
                [("q", 0), ("k", 0), ("v", 0)]
                + [(nm, c) for c in range(1, cfg.n_tc) for nm in ("k", "v")]
                + [("q", c) for c in range(1, cfg.n_tc)]
            )
            xts_cur = emit_xt_loads(0)
            drain_all(qkv_stream(0, xts_cur, lp=False, order=ramp_order))
            pending_out = None
            for b in range(B):
                xts_next = emit_xt_loads(b + 1) if b + 1 < B else None
                qs = qkv_stream(b + 1, xts_next) if xts_next is not None else None
                for ic in range(cfg.n_tc):
                    fillers = [pending_out, qs]
                    avs = emit_attention_jts(b, ic, fillers)
                    pending_out, qs = fillers
                    drain_all(pending_out)  # anything left from prev chunk
                    pending_out = out_stream(b, ic, avs)
                xts_cur = xts_next
                # spread remaining qkv work into the next batch boundary
                drain_all(qs)
            drain_all(pending_out)

    if split_waits:
        # walrus needs this; CoreSim chokes on the bare NOPs, so skip there
        split_sync_waits(nc)
    return nc


# ---------------------------------------------------------------------------
# Host entry point
# ---------------------------------------------------------------------------
_CACHE = {}


def _get_nc():
    if "nc" not in _CACHE:
        _CACHE["nc"] = build(FULL)
    return _CACHE["nc"]


def kernel(x: np.ndarray, W_q: np.ndarray, W_k: np.ndarray, W_v: np.ndarray) -> np.ndarray:
    from concourse.bass_utils import run_bass_kernel_spmd

    cfg = FULL
    B, S, D, E, HPC = cfg.B, cfg.S, cfg.D, cfg.E, cfg.HPC
    T, M = cfg.T, cfg.M
    H = HPC * N_CORES

    assert x.shape == (B, S, D) and W_q.shape == (H, E, D)

    bf16 = ml_dtypes.bfloat16
    xtT = np.ascontiguousarray(x.reshape(T, D).T).astype(bf16)  # [D, T]

    in_maps = []
    for c in range(N_CORES):
        m = {"xt": xtT}
        for nm, W in (("wq", W_q), ("wk", W_k), ("wv", W_v)):
            # pack 2 heads: [D, 128] with col h*E+e = W[2c+h][e, :]
            blk = W[HPC * c:HPC * (c + 1)]          # [HPC, E, D]
            wt = blk.reshape(M, D).T                # [D, M]
            m[nm] = np.ascontiguousarray(wt).astype(bf16)
        in_maps.append(m)

    kw = dict(_CACHE.get("run_kwargs", {}))
    res = run_bass_kernel_spmd(_get_nc(), in_maps, list(range(N_CORES)), **kw)
    _CACHE["last_res"] = res
    outs = [res.results[c]["out"] for c in range(N_CORES)]  # each [T, M]
    full = np.concatenate(outs, axis=1)                     # [T, H*E]
    return full.reshape(B, S, H * E).astype(np.float32, copy=False)
